# revision 16
# baseline (speedup 1.0000x reference)
"""Trainium2 Bass kernel for nn_PrecisionFocusedLoss.

Reference computation (per sample, logits (B,2) f32, targets (B,) int):
    ce   = -log_softmax(logits)[target]
    pred = argmax(logits)
    penalty: (t==1 & p==0) -> 1.0 ; (t==0 & p==1) -> 5.0 ; else 0.1
    loss = mean(ce * (1 + 3*penalty))

Math used here: with d = l1 - l0, s = 1 - 2t (+1 if t==0 else -1),
u = s*d, h = (u > 0):
    ce = softplus(u) = ln(1 + exp(u))
    weight = 1.3 + h*(8.7 + 6*s)        # 16 for FP, 4 for FN, 1.3 correct
    total  = 1.3*sum(sp) + 8.7*sum(h*sp) + 6*sum(h*sp*s)
The three sums (A, B1, B2) are computed per-partition with free
accumulation (ACT accum_out for A, scalar_tensor_tensor accum_out for
B1/B2) and finished on the host in float64.

Sharding: pure data-parallel, batch split across 8 NeuronCores; each
core reduces its 1/8 shard to [128, 3*n_acc] partials; host gathers.
"""

import numpy as np

import concourse.bass as bass
import concourse.bacc as bacc
import concourse.tile as tile
import concourse.mybir as mybir
from concourse.bass_utils import run_bass_kernel_spmd

# Problem constants (hardcoded per harness contract).
B_TOTAL = 8388608
N_CORES = 8
B_CORE = B_TOTAL // N_CORES  # 1048576
P = 128  # SBUF partitions

FP32 = mybir.dt.float32
BF16 = mybir.dt.bfloat16
I32 = mybir.dt.int32
AF = mybir.ActivationFunctionType
OP = mybir.AluOpType

# --- ACT table-set pinning ---------------------------------------------------
# The act-table-load inserter greedily picks the first act_func_set containing
# each activation's function, which ping-pongs between exp_and_others and
# natural_log (one ~1.3us table load per tile, twice per tile). All functions
# we use (Copy/Identity/Exp/Ln) live together in natural_log_exp_and_others,
# so lie to the chooser: claim they exist ONLY in that set (dict order, and
# hence act_func_set_id numbering, is preserved).
_PINNED_SET = "natural_log_exp_and_others"
_PINNED_FUNCS = {AF.Copy, AF.Identity, AF.Exp, AF.Ln}
_orig_get_tables = None


def _patched_get_activation_tables(arch):
    tabs = _orig_get_tables(arch)
    return {
        name: (funcs if name == _PINNED_SET else funcs - _PINNED_FUNCS)
        for name, funcs in tabs.items()
    }


def _install_table_patch():
    global _orig_get_tables
    if _orig_get_tables is None:
        _orig_get_tables = bacc.get_activation_tables
        bacc.get_activation_tables = _patched_get_activation_tables


def build_loss_nc(repeat: int = 1, loop_repeat: int = 0,
                  f_dma: int = 512, f_cmp: int = 512,
                  d_engine: str = "pool", s_engine: str = "act",
                  work_bufs: int = 3, io_bufs: int = 3,
                  t_first: bool = True, t_swdge: bool = False,
                  plan=None):
    """Build the single-core Bass program.

    f_dma: samples per partition-row per DMA tile.
    f_cmp: samples per partition-row per compute chunk (divides f_dma).
    d_engine: 'dve' or 'pool' -- engine for d = l1 - l0.
    s_engine: 'pool'/'act'/'dve' -- engine for s = 1 - 2t.
    plan: optional explicit tile plan [(f_dma, f_cmp, d_engine), ...]
          covering B_CORE // P samples per partition row; overrides
          f_dma/f_cmp/d_engine.
    """
    _install_table_patch()
    if plan is None:
        assert B_CORE % (P * f_dma) == 0 and f_dma % f_cmp == 0
        n_tiles = B_CORE // (P * f_dma)
        plan = [(f_dma, f_cmp, d_engine)] * n_tiles
    assert sum(fd for fd, _, _ in plan) * P == B_CORE
    assert all(fd % fc == 0 for fd, fc, _ in plan)
    n_acc = sum(fd // fc for fd, fc, _ in plan)  # accum columns per sum

    nc = bacc.Bacc(
        "TRN2",
        target_bir_lowering=False,
        debug=False,
        enable_asserts=False,
        num_devices=1,
    )

    logits = nc.dram_tensor("logits_f", (B_CORE * 2,), FP32,
                            kind="ExternalInput").ap()
    targets = nc.dram_tensor("targets", (B_CORE,), I32,
                             kind="ExternalInput").ap()
    partials = nc.dram_tensor("partials", (P, 3 * n_acc), FP32,
                              kind="ExternalOutput").ap()

    # Row-major per-partition layout: partition p owns a contiguous run of
    # B_CORE/P samples; tiles slice the free dim at arbitrary offsets.
    lg = logits.rearrange("(p f) -> p f", p=P)
    tg = targets.rearrange("(p f) -> p f", p=P)

    with tile.TileContext(nc) as tc:
        with (
            tc.tile_pool(name="lpool", bufs=io_bufs) as lpool,
            tc.tile_pool(name="tpool", bufs=io_bufs) as tpool,
            tc.tile_pool(name="work", bufs=work_bufs) as work,
            tc.tile_pool(name="acc", bufs=1) as accp,
        ):
            # One accumulator tile, three column regions (A | B1 | B2) so
            # a single DMA writes everything out at the end.
            acc = accp.tile([P, 3 * n_acc], FP32, tag="acc")

            def one_pass():
                k = 0
                sample_off = 0
                for (fd, fc, d_eng) in plan:
                    L = lpool.tile([P, 2 * fd], FP32, tag="L")
                    T = tpool.tile([P, fd], I32, tag="T")
                    lg_v = lg[:, 2 * sample_off:2 * (sample_off + fd)]
                    tg_v = tg[:, sample_off:sample_off + fd]
                    sample_off += fd
                    t_dma = nc.gpsimd.dma_start if t_swdge else nc.sync.dma_start
                    if t_first:
                        t_dma(out=T[:, :], in_=tg_v)
                        nc.sync.dma_start(out=L[:, :], in_=lg_v)
                    else:
                        nc.sync.dma_start(out=L[:, :], in_=lg_v)
                        t_dma(out=T[:, :], in_=tg_v)

                    for c in range(fd // fc):
                        cs = slice(c * fc, (c + 1) * fc)
                        l0 = L[:, 2 * c * fc:2 * (c + 1) * fc:2]
                        l1 = L[:, 2 * c * fc + 1:2 * (c + 1) * fc:2]

                        s = work.tile([P, fc], BF16, tag="s")
                        d = work.tile([P, fc], BF16, tag="d")
                        u = work.tile([P, fc], BF16, tag="u")
                        e = work.tile([P, fc], FP32, tag="e")
                        sp = work.tile([P, fc], FP32, tag="sp")
                        hsp = work.tile([P, fc], FP32, tag="hsp")
                        scr = work.tile([P, fc], BF16, tag="scr")

                        # s = 1 - 2t  (int32 -> bf16 affine)
                        if s_engine == "pool":
                            nc.gpsimd.tensor_scalar(s[:, :], T[:, cs], -2.0, 1.0,
                                                    op0=OP.mult, op1=OP.add)
                        elif s_engine == "dve":
                            nc.vector.tensor_scalar(s[:, :], T[:, cs], -2.0, 1.0,
                                                    op0=OP.mult, op1=OP.add)
                        else:
                            nc.scalar.activation(s[:, :], T[:, cs], AF.Copy,
                                                 bias=1.0, scale=-2.0)
                        # d = l1 - l0 (strided f32 reads)
                        if d_eng == "pool":
                            nc.gpsimd.tensor_tensor(d[:, :], l1, l0, OP.subtract)
                        else:
                            nc.vector.tensor_tensor(d[:, :], l1, l0, OP.subtract)
                        # u = s * d  (bf16 2x)
                        nc.vector.tensor_tensor(u[:, :], s[:, :], d[:, :], OP.mult)
                        # sp = ln(exp(u) + 1), accum A
                        nc.scalar.activation(e[:, :], u[:, :], AF.Exp)
                        nc.scalar.activation(sp[:, :], e[:, :], AF.Ln,
                                             bias=1.0,
                                             accum_out=acc[:, k:k + 1])
                        # hsp = (u > 0) * sp, accum B1
                        nc.vector.scalar_tensor_tensor(
                            hsp[:, :], u[:, :], 0.0, sp[:, :],
                            op0=OP.is_gt, op1=OP.mult,
                            accum_out=acc[:, n_acc + k:n_acc + k + 1])
                        # scr = hsp * s, accum B2
                        nc.vector.scalar_tensor_tensor(
                            scr[:, :], hsp[:, :], 0.0, s[:, :],
                            op0=OP.bypass, op1=OP.mult,
                            accum_out=acc[:, 2 * n_acc + k:2 * n_acc + k + 1])
                        k += 1

            if loop_repeat:
                with tc.For_i(0, loop_repeat, 1):
                    one_pass()
            else:
                for r in range(repeat):
                    one_pass()

            nc.sync.dma_start(out=partials[:, :], in_=acc[:, :])

    nc.compile()
    nc._n_acc = n_acc
    return nc


def _host_reduce(partial_list, n_acc: int) -> np.float32:
    """partial_list: per-core [P, 3*n_acc] f32 arrays -> scalar mean."""
    total = 0.0
    for p in partial_list:
        p64 = p.astype(np.float64)
        a = p64[:, 0:n_acc].sum()
        b1 = p64[:, n_acc:2 * n_acc].sum()
        b2 = p64[:, 2 * n_acc:3 * n_acc].sum()
        total += 1.3 * a + 8.7 * b1 + 6.0 * b2
    return np.float32(total / B_TOTAL)


def kernel(logits, targets, _build_kwargs: dict | None = None,
           _nc_cache: dict = {}):
    logits = np.ascontiguousarray(np.asarray(logits), dtype=np.float32)
    targets = np.asarray(targets)
    if targets.dtype != np.int32:
        targets = targets.astype(np.int32)
    targets = np.ascontiguousarray(targets)
    assert logits.shape == (B_TOTAL, 2) and targets.shape == (B_TOTAL,)

    key = tuple(sorted((_build_kwargs or {}).items()))
    if key not in _nc_cache:
        _nc_cache[key] = build_loss_nc(**(_build_kwargs or {}))
    nc = _nc_cache[key]

    lf = logits.reshape(N_CORES, B_CORE * 2)
    tf = targets.reshape(N_CORES, B_CORE)
    in_maps = [
        {"logits_f": lf[c], "targets": tf[c]}
        for c in range(N_CORES)
    ]
    res = run_bass_kernel_spmd(nc, in_maps, core_ids=list(range(N_CORES)))
    return _host_reduce([r["partials"] for r in res.results], nc._n_acc)


# revision 17
# speedup vs baseline: 1.3057x; 1.3057x over previous
"""Trainium2 Bass kernel for nn_PrecisionFocusedLoss.

Reference computation (per sample, logits (B,2) f32, targets (B,) int):
    ce   = -log_softmax(logits)[target]
    pred = argmax(logits)
    penalty: (t==1 & p==0) -> 1.0 ; (t==0 & p==1) -> 5.0 ; else 0.1
    loss = mean(ce * (1 + 3*penalty))

Math used here: with d = l1 - l0, s = 1 - 2t (+1 if t==0 else -1),
u = s*d, h = (u > 0):
    ce = softplus(u) = ln(1 + exp(u))
    weight = 1.3 + h*(8.7 + 6*s)        # 16 for FP, 4 for FN, 1.3 correct
    total  = 1.3*sum(sp) + 8.7*sum(h*sp) + 6*sum(h*sp*s)
The three sums (A, B1, B2) are computed per-partition with free
accumulation (ACT accum_out for A, scalar_tensor_tensor accum_out for
B1/B2) and finished on the host in float64.

Sharding: pure data-parallel, batch split across 8 NeuronCores; each
core reduces its 1/8 shard to [128, 3*n_acc] partials; host gathers.
"""

import numpy as np

import concourse.bass as bass
import concourse.bacc as bacc
import concourse.tile as tile
import concourse.mybir as mybir
from concourse.bass_utils import run_bass_kernel_spmd

# Problem constants (hardcoded per harness contract).
B_TOTAL = 8388608
N_CORES = 8
B_CORE = B_TOTAL // N_CORES  # 1048576
P = 128  # SBUF partitions

FP32 = mybir.dt.float32
BF16 = mybir.dt.bfloat16
I32 = mybir.dt.int32
AF = mybir.ActivationFunctionType
OP = mybir.AluOpType

# --- ACT table-set pinning ---------------------------------------------------
# The act-table-load inserter greedily picks the first act_func_set containing
# each activation's function, which ping-pongs between exp_and_others and
# natural_log (one ~1.3us table load per tile, twice per tile). All functions
# we use (Copy/Identity/Exp/Ln) live together in natural_log_exp_and_others,
# so lie to the chooser: claim they exist ONLY in that set (dict order, and
# hence act_func_set_id numbering, is preserved).
_PINNED_SET = "natural_log_exp_and_others"
_PINNED_FUNCS = {AF.Copy, AF.Identity, AF.Exp, AF.Ln}
_orig_get_tables = None


def _patched_get_activation_tables(arch):
    tabs = _orig_get_tables(arch)
    return {
        name: (funcs if name == _PINNED_SET else funcs - _PINNED_FUNCS)
        for name, funcs in tabs.items()
    }


def _install_table_patch():
    global _orig_get_tables
    if _orig_get_tables is None:
        _orig_get_tables = bacc.get_activation_tables
        bacc.get_activation_tables = _patched_get_activation_tables


def build_loss_nc(repeat: int = 1, loop_repeat: int = 0,
                  f_dma: int = 512, f_cmp: int = 512,
                  d_engine: str = "pool", s_engine: str = "act",
                  work_bufs: int = 3, io_bufs: int = 3,
                  t_first: bool = True, t_swdge: bool = False,
                  plan=None):
    """Build the single-core Bass program.

    f_dma: samples per partition-row per DMA tile.
    f_cmp: samples per partition-row per compute chunk (divides f_dma).
    d_engine: 'dve' or 'pool' -- engine for d = l1 - l0.
    s_engine: 'pool'/'act'/'dve' -- engine for s = 1 - 2t.
    plan: optional explicit tile plan [(f_dma, f_cmp, d_engine), ...]
          covering B_CORE // P samples per partition row; overrides
          f_dma/f_cmp/d_engine.
    """
    _install_table_patch()
    if plan is None:
        assert B_CORE % (P * f_dma) == 0 and f_dma % f_cmp == 0
        n_tiles = B_CORE // (P * f_dma)
        plan = [(f_dma, f_cmp, d_engine)] * n_tiles
    assert sum(fd for fd, _, _ in plan) * P == B_CORE
    assert all(fd % fc == 0 for fd, fc, _ in plan)
    n_acc = sum(fd // fc for fd, fc, _ in plan)  # accum columns per sum

    nc = bacc.Bacc(
        "TRN2",
        target_bir_lowering=False,
        debug=False,
        enable_asserts=False,
        num_devices=1,
    )

    logits = nc.dram_tensor("logits_f", (B_CORE * 2,), FP32,
                            kind="ExternalInput").ap()
    targets = nc.dram_tensor("targets", (B_CORE,), I32,
                             kind="ExternalInput").ap()
    partials = nc.dram_tensor("partials", (P, 3 * n_acc), FP32,
                              kind="ExternalOutput").ap()

    # Row-major per-partition layout: partition p owns a contiguous run of
    # B_CORE/P samples; tiles slice the free dim at arbitrary offsets.
    lg = logits.rearrange("(p f) -> p f", p=P)
    tg = targets.rearrange("(p f) -> p f", p=P)

    with tile.TileContext(nc) as tc:
        with (
            tc.tile_pool(name="lpool", bufs=io_bufs) as lpool,
            tc.tile_pool(name="tpool", bufs=io_bufs) as tpool,
            tc.tile_pool(name="work", bufs=work_bufs) as work,
            tc.tile_pool(name="acc", bufs=1) as accp,
        ):
            # One accumulator tile, three column regions (A | B1 | B2) so
            # a single DMA writes everything out at the end.
            acc = accp.tile([P, 3 * n_acc], FP32, tag="acc")

            def one_pass():
                k = 0
                sample_off = 0
                for (fd, fc, d_eng) in plan:
                    L = lpool.tile([P, 2 * fd], FP32, tag="L")
                    T = tpool.tile([P, fd], I32, tag="T")
                    lg_v = lg[:, 2 * sample_off:2 * (sample_off + fd)]
                    tg_v = tg[:, sample_off:sample_off + fd]
                    sample_off += fd
                    t_dma = nc.gpsimd.dma_start if t_swdge else nc.sync.dma_start
                    if t_first:
                        t_dma(out=T[:, :], in_=tg_v)
                        nc.sync.dma_start(out=L[:, :], in_=lg_v)
                    else:
                        nc.sync.dma_start(out=L[:, :], in_=lg_v)
                        t_dma(out=T[:, :], in_=tg_v)

                    for c in range(fd // fc):
                        cs = slice(c * fc, (c + 1) * fc)
                        l0 = L[:, 2 * c * fc:2 * (c + 1) * fc:2]
                        l1 = L[:, 2 * c * fc + 1:2 * (c + 1) * fc:2]

                        s = work.tile([P, fc], BF16, tag="s")
                        d = work.tile([P, fc], BF16, tag="d")
                        u = work.tile([P, fc], BF16, tag="u")
                        e = work.tile([P, fc], FP32, tag="e")
                        sp = work.tile([P, fc], FP32, tag="sp")
                        hsp = work.tile([P, fc], FP32, tag="hsp")
                        scr = work.tile([P, fc], BF16, tag="scr")

                        # s = 1 - 2t  (int32 -> bf16 affine)
                        s_eng = s_engine
                        if s_eng == "alt":
                            s_eng = "pool" if k % 2 == 0 else "act"
                        if s_eng == "pool":
                            nc.gpsimd.tensor_scalar(s[:, :], T[:, cs], -2.0, 1.0,
                                                    op0=OP.mult, op1=OP.add)
                        elif s_eng == "dve":
                            nc.vector.tensor_scalar(s[:, :], T[:, cs], -2.0, 1.0,
                                                    op0=OP.mult, op1=OP.add)
                        else:
                            nc.scalar.activation(s[:, :], T[:, cs], AF.Copy,
                                                 bias=1.0, scale=-2.0)
                        # d = l1 - l0 (strided f32 reads)
                        if d_eng == "pool":
                            nc.gpsimd.tensor_tensor(d[:, :], l1, l0, OP.subtract)
                        else:
                            nc.vector.tensor_tensor(d[:, :], l1, l0, OP.subtract)
                        # u = s * d  (bf16 2x)
                        nc.vector.tensor_tensor(u[:, :], s[:, :], d[:, :], OP.mult)
                        # sp = ln(exp(u) + 1), accum A
                        nc.scalar.activation(e[:, :], u[:, :], AF.Exp)
                        nc.scalar.activation(sp[:, :], e[:, :], AF.Ln,
                                             bias=1.0,
                                             accum_out=acc[:, k:k + 1])
                        # hsp = (u > 0) * sp, accum B1
                        nc.vector.scalar_tensor_tensor(
                            hsp[:, :], u[:, :], 0.0, sp[:, :],
                            op0=OP.is_gt, op1=OP.mult,
                            accum_out=acc[:, n_acc + k:n_acc + k + 1])
                        # scr = hsp * s, accum B2
                        nc.vector.scalar_tensor_tensor(
                            scr[:, :], hsp[:, :], 0.0, s[:, :],
                            op0=OP.bypass, op1=OP.mult,
                            accum_out=acc[:, 2 * n_acc + k:2 * n_acc + k + 1])
                        k += 1

            if loop_repeat:
                with tc.For_i(0, loop_repeat, 1):
                    one_pass()
            else:
                for r in range(repeat):
                    one_pass()

            nc.sync.dma_start(out=partials[:, :], in_=acc[:, :])

    nc.compile()
    nc._n_acc = n_acc
    return nc


def _host_reduce(partial_list, n_acc: int) -> np.float32:
    """partial_list: per-core [P, 3*n_acc] f32 arrays -> scalar mean."""
    total = 0.0
    for p in partial_list:
        p64 = p.astype(np.float64)
        a = p64[:, 0:n_acc].sum()
        b1 = p64[:, n_acc:2 * n_acc].sum()
        b2 = p64[:, 2 * n_acc:3 * n_acc].sum()
        total += 1.3 * a + 8.7 * b1 + 6.0 * b2
    return np.float32(total / B_TOTAL)


def kernel(logits, targets, _build_kwargs: dict | None = None,
           _nc_cache: dict = {}):
    logits = np.ascontiguousarray(np.asarray(logits), dtype=np.float32)
    targets = np.asarray(targets)
    if targets.dtype != np.int32:
        targets = targets.astype(np.int32)
    targets = np.ascontiguousarray(targets)
    assert logits.shape == (B_TOTAL, 2) and targets.shape == (B_TOTAL,)

    key = tuple(sorted((_build_kwargs or {}).items()))
    if key not in _nc_cache:
        _nc_cache[key] = build_loss_nc(**(_build_kwargs or {}))
    nc = _nc_cache[key]

    lf = logits.reshape(N_CORES, B_CORE * 2)
    tf = targets.reshape(N_CORES, B_CORE)
    in_maps = [
        {"logits_f": lf[c], "targets": tf[c]}
        for c in range(N_CORES)
    ]
    res = run_bass_kernel_spmd(nc, in_maps, core_ids=list(range(N_CORES)))
    return _host_reduce([r["partials"] for r in res.results], nc._n_acc)
